# revision 64
# baseline (speedup 1.0000x reference)
"""Trainium2 Bass kernel for nn_LocalGeometryLoss (threshold-mask rewrite).

Reference semantics (fp32):
    hp = l2norm(hidden_previous)                    # [8192, 768]
    sim = hp @ hp.T
    nbr = top_k(sim, 6)[:, 1:]                      # 5 nearest (self dropped)
    e[i,k] = +1 if labels_prev[i]==labels_prev[nbr[i,k]] else -1
    hc = l2norm(hidden_current)                     # [4096, 768]
    d2[i,j] = max(|hc_i|^2 + |hc_j|^2 - 2 hc_i.hc_j, 0)
    loss = 0.5 * sum_{i<4096, nbr j<4096} e * d2[i, nbr] / 4096^2

Strategy (per core, 512 of the 4096 contributing rows): select neighbors
by VALUE THRESHOLD instead of recovering top-k indices, then reduce the
masked loss densely.  No MaxIndex rescans, no indirect gathers, no
neighbor table.

  phase A: prev-Gram row-block [512, 8192] via fp8 DoubleRow matmuls
           (2 k-tiles contracted per instruction, 0.5 cyc/row) into
           [128,1024] PSUM tiles, RIGHT half (j>=4096) first (feed
           order interleaves right/left chunks).  Right half: DVE
           max8-scans the fp32 PSUM directly (first group split
           512-wide for an early start).  Left half: Activation only
           COPIES sims to SBUF fp16 (simL); candidates come from an
           fp16 TT-max fold tree (DVE 2x mode, w=4 windows
           {j, j+1024, j+2048, j+3072}) whose first level needs only
           left groups 0/1 and fills DVE's feed gaps.  fp16 (not bf16!)
           keeps the 5th/6th-value tie-collapse negligible.  Merged
           candidates give t'_i = midpoint of the 5th/6th neighbor
           values; the mask (fp16 simL > t'_i, DVE 4x tensor_scalar)
           is written over simL in place.
  phase B (overlaps A): cur-Gram via AUGMENTED fp8 vectors - feature
           767 is replaced by constants (lhsT: 16, rhs: 8) with the rhs
           hcn part negated, so PSUM directly holds 64*d2' (d2 over 767
           features) with no affine pass; Activation copies it to SBUF
           bf16 (d2t) after the simL copies.
  tail:    Q = mask * d2 (1024-wide tensor_tensor, split DVE 2x / Pool);
           G[cls, j] += onehot(l_i)^T Q per 512-chunk (bf16 matmuls,
           PSUM-accumulated over the 4 row-tiles); gw = G * W where
           W[cls,j] = 2*[l_j = cls]-1 (host fp8, odd chunks multiplied
           by DVE straight off G PSUM, even via Act copy + Pool);
           per-chunk full reduces stream straight to DRAM and the host
           sums/scales the 8x8 partials.  sum(G*W) == sum mask*e*d2.

Approximations (validated in numpy, rel-err 3.9e-3 vs the 2e-2 gate):
  * raw (unnormalized) fp8 prev vectors for selection - row scale never
    changes a row top-k; column jitter is unbiased selection noise
    (inherited from the previous kernel).
  * fp32 right-half candidates + fp16 left-half pooled candidates +
    fp16 mask: inclusion flips only for sub-ulp 5th/6th gaps.  bf16
    candidates would collapse v5==v6 ties and bias the loss ~-1.6%;
    fp16's 4x finer ulp brings that to ~1e-4 (measured).  w=4 pooling
    adds ~1% extra-neighbor rows - unbiased.
  * augmented fp8 cur vectors (767 features + constant): d2 noise
    ~0.3%, unbiased across ~20k signed terms; self term ~0 by design.
  * no Relu clamp on d2 (only affects the ~0 self term).

Cost-model facts this layout is built around (v1 instruction_cost.rs):
  * matmul = out_free x 0.4167ns x cyc/row; fp8 DoubleRow = 0.5 cyc/row
    AND contracts 2 k-tiles per instruction (4x the baseline rate).
  * only DVE can max; Pool (GPSIMD) cannot touch PSUM and only supports
    mult/add TensorTensor, so every sim exits PSUM through DVE max8
    scans or Activation copies (~1 elem/ns each) - that pass is the
    floor and DVE runs ~95% saturated start to finish.
  * DVE bf16 fast modes: tensor_scalar 4x, tensor_tensor 2x.
  * DMA costs per-partition-bytes x 0.3855ns on the issuing engine.

Measured (CoreSim cost model, 8-core axon): 58507 ns vs 123963 ns for
the session-start baseline (2.12x), relative error 3.3e-3.  DVE busy
dropped from 52.5us to 44.9us by replacing the left-half fp32 max8
PSUM scans with the fp16 fold tree (2x TT mode).
"""

import numpy as np
import ml_dtypes

import concourse.bass as bass
import concourse.bacc as bacc
import concourse.mybir as mybir
from concourse import tile
from concourse.bass_utils import run_bass_kernel_spmd

FP = mybir.dt.float32
BF = mybir.dt.bfloat16
F8 = mybir.dt.float8e4
F16 = mybir.dt.float16
ACT = mybir.ActivationFunctionType
ALU = mybir.AluOpType
DR = mybir.MatmulPerfMode.DoubleRow

B_PREV = 8192
B_CURR = 4096
D = 768
WEIGHT = 0.5
N_CORES = 8
ROWS_PER_CORE = B_CURR // N_CORES          # 512
M_TILES = ROWS_PER_CORE // 128             # 4
KC = D // 128                              # 6 k-tiles (3 DoubleRow steps)
WA = 1024                                  # phase-A chunk width
A_CHUNKS = B_PREV // WA                    # 8  (4 left + 4 right)
NCH = 512                                  # phase-B chunk width
C_CHUNKS = B_CURR // NCH                   # 8
NCLS = 100                                 # label classes
D2SCALE = 64.0                             # PSUM holds 64*d2

_CACHE = {}


def _build():
    nc = bacc.Bacc("TRN2", target_bir_lowering=False, debug=False,
                   num_devices=N_CORES, num_swdge_queues=4)

    hpT_d = nc.dram_tensor("hpT", [D, B_PREV], F8, kind="ExternalInput").ap()
    lhsTp_d = nc.dram_tensor("lhsTp", [D, ROWS_PER_CORE], F8,
                             kind="ExternalInput").ap()
    hcnT_d = nc.dram_tensor("hcnT", [D, B_CURR], F8, kind="ExternalInput").ap()
    lhsTc_d = nc.dram_tensor("lhsTc", [D, ROWS_PER_CORE], F8,
                             kind="ExternalInput").ap()
    u_d = nc.dram_tensor("u", [M_TILES, 128, NCLS], BF,
                         kind="ExternalInput").ap()
    w_d = nc.dram_tensor("w", [NCLS, B_CURR], F8, kind="ExternalInput").ap()

    partial = nc.dram_tensor("partial", [1, C_CHUNKS], FP,
                             kind="ExternalOutput").ap()

    hpT_r = hpT_d.rearrange("(kc p) j -> p kc j", p=128)
    hcnT_r = hcnT_d.rearrange("(kc p) j -> p kc j", p=128)

    with tile.TileContext(nc) as tc:
        sb = tc.alloc_tile_pool(name="sb", bufs=1)
        gwp = tc.alloc_tile_pool(name="gwp", bufs=2)
        pspC = tc.alloc_tile_pool(name="pspC", bufs=2, space="PSUM")
        pspA = tc.alloc_tile_pool(name="pspA", bufs=3, space="PSUM")

        # ---- persistent tiles ----
        hpT = sb.tile([128, KC, B_PREV], F8)
        lhsTp = sb.tile([128, KC, ROWS_PER_CORE], F8)
        hcnT = sb.tile([128, KC, B_CURR], F8)
        lhsTc = sb.tile([128, KC, ROWS_PER_CORE], F8)
        u_sb = sb.tile([128, M_TILES, NCLS], BF)
        w_sb = sb.tile([NCLS, B_CURR], F8)
        simL = sb.tile([128, M_TILES, B_CURR], F16)   # becomes mask in place
        cands = sb.tile([128, M_TILES, 48], FP)
        f1a = sb.tile([128, M_TILES, 1024], F16)
        f1b = sb.tile([128, 1024], F16)
        f2t = sb.tile([128, 1024], F16)
        v8 = sb.tile([128, M_TILES, 8], FP)
        thr = sb.tile([128, M_TILES], FP)
        tp = sb.tile([1, C_CHUNKS], FP)

        # ---- DMA: Pool gets lhsTp + first half of hcnT + lhsTc;
        #      SP streams hpT, hcnT tail, U, W, then phase-B d2 chunks.
        lhsTp_r = lhsTp_d.rearrange("(kc p) m -> p kc m", p=128)
        for k in range(KC // 2):
            nc.gpsimd.dma_start(lhsTp[:, 2 * k:2 * k + 2, :],
                                lhsTp_r[:, 2 * k:2 * k + 2, :])
        for c in range(C_CHUNKS // 2):
            nc.gpsimd.dma_start(hcnT[:, :, NCH * c:NCH * (c + 1)],
                                hcnT_r[:, :, NCH * c:NCH * (c + 1)])
        nc.gpsimd.dma_start(lhsTc[:],
                            lhsTc_d.rearrange("(kc p) m -> p kc m", p=128))
        for n in (9,):
            nc.scalar.dma_start(hpT[:, :, NCH * n:NCH * (n + 1)],
                                hpT_r[:, :, NCH * n:NCH * (n + 1)])
        for n in (8, 10, 0, 11, 1, 12, 2, 13, 3, 14, 4, 15, 5, 6, 7):
            nc.sync.dma_start(hpT[:, :, NCH * n:NCH * (n + 1)],
                              hpT_r[:, :, NCH * n:NCH * (n + 1)])
        for c in range(C_CHUNKS // 2, C_CHUNKS):
            nc.sync.dma_start(hcnT[:, :, NCH * c:NCH * (c + 1)],
                              hcnT_r[:, :, NCH * c:NCH * (c + 1)])
        nc.sync.dma_start(u_sb[:], u_d.rearrange("m p c -> p m c"))
        nc.sync.dma_start(w_sb[:], w_d)

        # ---- phases A+B interleaved ----
        # Persistent d2 / q tiles: phase-B products are computed as soon as
        # their PSUM chunks exist (overlapping the DVE-bound phase A); the
        # Q products for m-tiles 0..2 also run early, m-tile 3's Q and the
        # G/gw/reduce chain form the tail once the last threshold is known.
        d2t = sb.tile([128, M_TILES, C_CHUNKS * NCH], BF)
        qt = sb.tile([128, M_TILES, C_CHUNKS * NCH], BF)

        # A: right half first - DVE max8-scans fp32 PSUM (first right
        # group split 512-wide via the psC pool for an early start); left
        # half is only copied to fp16 simL by Activation.  PE production
        # interleaves right/left groups to match the interleaved feed.

        nc.gpsimd.memset(cands[:, 1:, 0:8], -1e30)

        def right_group(rg):
            # cands slots: rg 0 -> 0:16 (split for m0), rg 1..3 -> 16+8*rg
            for m in range(M_TILES):
                if rg == 0 and m == 0:
                    for h in range(2):
                        ps0 = pspC.tile([128, NCH], FP, tag="psC",
                                        name="psA0")
                        j0 = B_CURR + NCH * h
                        for k in range(KC // 2):
                            nc.tensor.matmul(
                                ps0[:], lhsTp[:, 2 * k:2 * k + 2, 0:128],
                                hpT[:, 2 * k:2 * k + 2, j0:j0 + NCH],
                                start=(k == 0), stop=(k == KC // 2 - 1),
                                perf_mode=DR)
                        nc.vector.max(out=cands[:, 0, 8 * h:8 * (h + 1)],
                                      in_=ps0[:])
                    continue
                ps = pspA.tile([128, WA], FP, tag="psA", name="psA")
                for h in range(2):
                    j0 = B_CURR + WA * rg + NCH * h
                    for k in range(KC // 2):
                        nc.tensor.matmul(
                            ps[:, NCH * h:NCH * (h + 1)],
                            lhsTp[:, 2 * k:2 * k + 2, 128 * m:128 * (m + 1)],
                            hpT[:, 2 * k:2 * k + 2, j0:j0 + NCH],
                            start=(k == 0), stop=(k == KC // 2 - 1),
                            perf_mode=DR)
                slot = 8 if rg == 0 else 8 * (rg + 1)
                nc.vector.max(out=cands[:, m, slot:slot + 8], in_=ps[:])

        def left_group(lg):
            for m in range(M_TILES):
                ps = pspA.tile([128, WA], FP, tag="psA", name="psA")
                for h in range(2):
                    j0 = WA * lg + NCH * h
                    for k in range(KC // 2):
                        nc.tensor.matmul(
                            ps[:, NCH * h:NCH * (h + 1)],
                            lhsTp[:, 2 * k:2 * k + 2, 128 * m:128 * (m + 1)],
                            hpT[:, 2 * k:2 * k + 2, j0:j0 + NCH],
                            start=(k == 0), stop=(k == KC // 2 - 1),
                            perf_mode=DR)
                nc.scalar.copy(simL[:, m, WA * lg:WA * (lg + 1)], ps[:])

        right_group(0)
        left_group(0)
        right_group(1)
        left_group(1)
        # first fold level (needs only left groups 0,1) fills DVE gaps
        def fold_a(m):
            nc.vector.tensor_tensor(out=f1a[:, m, :], in0=simL[:, m, :1024],
                                    in1=simL[:, m, 1024:2048], op=ALU.max)
        fold_a(0)
        right_group(2)
        left_group(2)
        fold_a(1)
        fold_a(2)
        right_group(3)
        left_group(3)
        fold_a(3)

        def finish_m(m):
            # second fold level (left groups 2,3 + combine) and max8
            nc.vector.tensor_tensor(out=f1b[:], in0=simL[:, m, 2048:3072],
                                    in1=simL[:, m, 3072:4096], op=ALU.max)
            nc.vector.tensor_tensor(out=f2t[:], in0=f1a[:, m, :],
                                    in1=f1b[:], op=ALU.max)
            nc.vector.max(out=cands[:, m, 40:48], in_=f2t[:])
            nc.vector.max(out=v8[:, m, :], in_=cands[:, m, :])
            nc.vector.tensor_tensor(out=thr[:, m:m + 1],
                                    in0=v8[:, m, 5:6], in1=v8[:, m, 6:7],
                                    op=ALU.add)
            nc.vector.tensor_scalar(out=thr[:, m:m + 1], in0=thr[:, m:m + 1],
                                    scalar1=0.5, scalar2=None, op0=ALU.mult)
            nc.vector.tensor_scalar(out=simL[:, m, :], in0=simL[:, m, :],
                                    scalar1=thr[:, m:m + 1], scalar2=None,
                                    op0=ALU.is_gt)

        for m in range(M_TILES):
            finish_m(m)

        # B: cur-Gram + d2 copies (Act, after all simL copies)
        for c in range(C_CHUNKS):
            for m in range(M_TILES):
                psc = pspC.tile([128, NCH], FP, tag="psC", name="psC")
                for k in range(KC // 2):
                    nc.tensor.matmul(
                        psc[:],
                        lhsTc[:, 2 * k:2 * k + 2, 128 * m:128 * (m + 1)],
                        hcnT[:, 2 * k:2 * k + 2, NCH * c:NCH * (c + 1)],
                        start=(k == 0), stop=(k == KC // 2 - 1),
                        perf_mode=DR)
                nc.scalar.copy(d2t[:, m, NCH * c:NCH * (c + 1)], psc[:])
        pspA.release()
        pspG = tc.alloc_tile_pool(name="pspG", bufs=2, space="PSUM")

        # Q (1024-wide, split DVE/Pool), G per 512-chunk, gw split:
        # odd chunks DVE-TT straight off G's PSUM, even via Act copy+Pool TT
        for c2 in range(C_CHUNKS // 2):
            for m in range(M_TILES):
                i = c2 * M_TILES + m
                eng = nc.gpsimd if i % 3 == 2 else nc.vector
                eng.tensor_tensor(
                    out=qt[:, m, WA * c2:WA * (c2 + 1)],
                    in0=simL[:, m, WA * c2:WA * (c2 + 1)],
                    in1=d2t[:, m, WA * c2:WA * (c2 + 1)], op=ALU.mult)
            for cc in (2 * c2, 2 * c2 + 1):
                gpsum = pspG.tile([NCLS, NCH], FP, tag="psG", name="psG")
                for m in range(M_TILES):
                    nc.tensor.matmul(gpsum[:], u_sb[:, m, :],
                                     qt[:, m, NCH * cc:NCH * (cc + 1)],
                                     start=(m == 0), stop=(m == M_TILES - 1))
                if cc % 2 == 1:
                    gw = gwp.tile([NCLS, NCH], BF, tag="gw", name="gw")
                    nc.vector.tensor_tensor(
                        out=gw[:], in0=gpsum[:],
                        in1=w_sb[:, NCH * cc:NCH * (cc + 1)], op=ALU.mult)
                else:
                    gsb = gwp.tile([NCLS, NCH], BF, tag="gsb", name="gsb")
                    nc.scalar.copy(gsb[:], gpsum[:])
                    gw = gwp.tile([NCLS, NCH], BF, tag="gw", name="gw")
                    nc.gpsimd.tensor_tensor(
                        out=gw[:], in0=gsb[:],
                        in1=w_sb[:, NCH * cc:NCH * (cc + 1)], op=ALU.mult)
                nc.gpsimd.tensor_reduce(out=tp[:, cc:cc + 1], in_=gw[:],
                                        axis=mybir.AxisListType.XYZWC,
                                        op=ALU.add)
                nc.sync.dma_start(partial[:, cc:cc + 1], tp[:, cc:cc + 1])


        for p in (pspG, pspC, gwp, sb):
            p.release()

    nc.compile()
    return nc


def _get_nc():
    if "nc" not in _CACHE:
        _CACHE["nc"] = _build()
    return _CACHE["nc"]


def _in_maps(inputs):
    f8 = ml_dtypes.float8_e4m3
    bf = ml_dtypes.bfloat16
    hp = np.asarray(inputs["hidden_previous"], dtype=np.float32)
    hc = np.asarray(inputs["hidden_current"], dtype=np.float32)
    lp = np.asarray(inputs["labels_previous"]).astype(np.int64)

    hpT = np.ascontiguousarray(hp.T.astype(f8))            # [768, 8192]
    hcn = hc / np.maximum(np.linalg.norm(hc, axis=1, keepdims=True), 1e-12)

    # augmented cur-side: feature 767 replaced by constants so the Gram
    # directly yields 64*d2 (see module docstring)
    lhsc = np.empty((B_CURR, D), dtype=np.float32)
    lhsc[:, :767] = 16.0 * hcn[:, :767]
    lhsc[:, 767] = 16.0
    rhsc = np.empty((B_CURR, D), dtype=np.float32)
    rhsc[:, :767] = -8.0 * hcn[:, :767]
    rhsc[:, 767] = 8.0
    hcnT = np.ascontiguousarray(rhsc.T.astype(f8))         # [768, 4096]
    lhscT = np.ascontiguousarray(lhsc.T.astype(f8))        # [768, 4096]

    lpc = lp[:B_CURR]
    W = np.full((NCLS, B_CURR), -1.0, dtype=np.float32)
    W[lpc, np.arange(B_CURR)] = 1.0
    W = W.astype(f8)

    in_maps = []
    for core in range(N_CORES):
        r0 = core * ROWS_PER_CORE
        lrows = lp[r0:r0 + ROWS_PER_CORE]
        U = np.zeros((ROWS_PER_CORE, NCLS), dtype=np.float32)
        U[np.arange(ROWS_PER_CORE), lrows] = 1.0
        in_maps.append({
            "hpT": hpT,
            "lhsTp": np.ascontiguousarray(hpT[:, r0:r0 + ROWS_PER_CORE]),
            "hcnT": hcnT,
            "lhsTc": np.ascontiguousarray(lhscT[:, r0:r0 + ROWS_PER_CORE]),
            "u": U.reshape(M_TILES, 128, NCLS).astype(bf),
            "w": W,
        })
    return in_maps


def _combine(out):
    total = np.float32(0.0)
    for c in range(N_CORES):
        total += out.results[c]["partial"].sum(dtype=np.float32)
    scale = np.float32(WEIGHT / (D2SCALE * B_CURR * B_CURR))
    return np.asarray(total * scale, dtype=np.float32)


def kernel(hidden_current, hidden_previous, labels_current, labels_previous,
           _want_debug=False):
    nc = _get_nc()
    in_maps = _in_maps({
        "hidden_current": hidden_current,
        "hidden_previous": hidden_previous,
        "labels_current": labels_current,
        "labels_previous": labels_previous,
    })
    out = run_bass_kernel_spmd(nc, in_maps, list(range(N_CORES)))
    result = _combine(out)
    if _want_debug:
        return result, out
    return result
